# Initial kernel scaffold
#
"""Trainium2 Bass kernel for nn_Block_21809843929850 (topk_masking).

Math (after removing dead code in the reference):
  The reference scatters s_out (attention output) into `out` and then
  immediately overwrites the exact same index set with `rev`, so the whole
  q/k/v/attention branch never reaches the output.  What remains is:

    rscore = x @ router_w.T            (router_b shifts all scores equally ->
                                        irrelevant for the top-k *set*)
    M[i,j] = 1 iff rscore[i,j] in top-512 of row i
    h1     = LN(x) * g1 + b1
    xn     = x + M * reverse_seq(h1)        (out[i,j] = M[i,j]*h1[i, L-1-j])
    h2     = LN(xn) * g2 + b2
    y      = xn + gelu_tanh(h2 @ fc_w.T + fc_b) @ proj_w.T + proj_b

Sharding: data-parallel over batch (8 rows -> 8 cores); weights replicated.
Weights for the two MLP matmuls are passed host-transposed+bf16 ([in, out]
layout) so both matmuls contract over the partition dim with no on-device
weight transposes.  All LN/mask/elementwise math runs in fp32.
"""

import sys

sys.path.insert(0, "/opt/trn_rl_repo")

import math

import numpy as np
import ml_dtypes

import concourse.bass as bass
import concourse.mybir as mybir
import concourse.bass_isa as bass_isa
from concourse import bacc
from concourse import bass_utils
from concourse.tile import TileContext

F32 = mybir.dt.float32
BF16 = mybir.dt.bfloat16
AF = mybir.ActivationFunctionType
ALU = mybir.AluOpType

B, L, D = 8, 2048, 1024
DF = 4 * D                     # 4096
K = math.ceil(L * 0.25)        # 512 (top-k size)
NT = L // 128                  # 16 token tiles of 128
TOK_BLK = 512                  # tokens per MLP block
NBLK = L // TOK_BLK            # 4
N_BISECT = 30
EPS = 1e-5

_cached = {}


def build_program(use_g1b1: bool, use_pb: bool):
    key = (use_g1b1, use_pb)
    if key in _cached:
        return _cached[key]

    nc = bacc.Bacc("TRN2", target_bir_lowering=False, debug=False)

    # ---- DRAM I/O ----
    x_d = nc.dram_tensor("x", [L, D], F32, kind="ExternalInput")
    rwb_d = nc.dram_tensor("rwb", [128, D], F32, kind="ExternalInput")
    ln1g_d = nc.dram_tensor("ln1gb", [2, 128, D], F32, kind="ExternalInput")
    ln2_d = nc.dram_tensor("ln2", [2, D], F32, kind="ExternalInput")   # [g;b]
    fcwT_d = nc.dram_tensor("fcwT", [D, DF], BF16, kind="ExternalInput")
    fcb_d = nc.dram_tensor("fcb", [DF], F32, kind="ExternalInput")
    pwT_d = nc.dram_tensor("pwT", [DF, D], BF16, kind="ExternalInput")
    pbb_d = nc.dram_tensor("pbb", [128, D], F32, kind="ExternalInput")
    aux_d = nc.dram_tensor("aux", [3, 128, 128], F32, kind="ExternalInput")
    # aux[0] = ones(128,128), aux[1] = J (anti-diagonal), aux[2] = identity
    out_d = nc.dram_tensor("out", [L, D], F32, kind="ExternalOutput")

    with TileContext(nc) as tc:
        with (
            tc.tile_pool(name="persist", bufs=1) as persist,
            tc.tile_pool(name="xpool", bufs=1) as xpool,
            tc.tile_pool(name="work", bufs=3) as work,
            tc.tile_pool(name="tiny", bufs=4) as tiny,
            tc.tile_pool(name="wstream", bufs=4) as wstream,
            tc.tile_pool(name="gpool", bufs=1) as gpool,
            tc.tile_pool(name="h2pool", bufs=2) as h2pool,
            tc.tile_pool(name="ypool", bufs=3) as ypool,
            tc.tile_pool(name="psum", bufs=2, space="PSUM") as psum,
            tc.tile_pool(name="psum_y", bufs=2, space="PSUM") as psum_y,
            tc.tile_pool(name="psum_tp", bufs=2, space="PSUM") as psum_tp,
        ):
            # ---- persistent small tensors ----
            ones_sb = persist.tile([128, 128], F32, tag="ones")
            nc.sync.dma_start(ones_sb, aux_d[0])
            J_sb = persist.tile([128, 128], F32, tag="J")
            nc.sync.dma_start(J_sb, aux_d[1])
            ident_sb = persist.tile([128, 128], F32, tag="ident")
            nc.sync.dma_start(ident_sb, aux_d[2])
            rwb_sb = persist.tile([128, D], F32, tag="rwb")
            nc.sync.dma_start(rwb_sb, rwb_d)
            # ln2 g/b as [128, 8] (d = k*128 + p)
            ln2g_sb = persist.tile([128, D // 128], F32, tag="ln2g")
            nc.sync.dma_start(ln2g_sb, ln2_d[0].rearrange("(ko p) -> p ko", p=128))
            ln2b_sb = persist.tile([128, D // 128], F32, tag="ln2b")
            nc.sync.dma_start(ln2b_sb, ln2_d[1].rearrange("(ko p) -> p ko", p=128))
            fcb_sb = persist.tile([128, DF // 128], F32, tag="fcb")
            nc.sync.dma_start(fcb_sb, fcb_d.rearrange("(c p) -> p c", p=128))
            if use_g1b1:
                g1_sb = persist.tile([128, D], F32, tag="g1")
                nc.sync.dma_start(g1_sb, ln1g_d[0])
                b1_sb = persist.tile([128, D], F32, tag="b1")
                nc.sync.dma_start(b1_sb, ln1g_d[1])
            if use_pb:
                pb_sb = persist.tile([128, D], F32, tag="pb")
                nc.sync.dma_start(pb_sb, pbb_d)
            eps_sb = persist.tile([128, 1], F32, tag="eps")
            nc.vector.memset(eps_sb, EPS)

            # proj_wT resident: [128, 32, 1024] bf16 (of = c*128 + p)
            pwT_sb = persist.tile([128, DF // 128, D], BF16, tag="pwT")
            nc.sync.dma_start(pwT_sb, pwT_d.rearrange("(c p) o -> p c o", p=128))

            # ---- load x ----
            x_tiles = []
            for t in range(NT):
                xt = xpool.tile([128, D], F32, tag=f"x{t}")
                nc.sync.dma_start(xt, x_d[t * 128:(t + 1) * 128, :])
                x_tiles.append(xt)

            # ---- router scores ----
            rs = persist.tile([128, NT], F32, tag="rs")
            for t in range(NT):
                trash = work.tile([128, D], F32, tag="rtrash")
                nc.vector.tensor_tensor_reduce(
                    out=trash, in0=x_tiles[t], in1=rwb_sb, scale=1.0, scalar=0.0,
                    op0=ALU.mult, op1=ALU.add, accum_out=rs[:, t:t + 1],
                )

            # ---- top-k threshold by bisection ----
            lo = persist.tile([128, 1], F32, tag="lo")
            hi = persist.tile([128, 1], F32, tag="hi")
            pm = tiny.tile([128, 1], F32, tag="pm")
            nc.vector.tensor_reduce(pm, rs, axis=mybir.AxisListType.X, op=ALU.max)
            nc.gpsimd.partition_all_reduce(hi, pm, channels=128,
                                           reduce_op=bass_isa.ReduceOp.max)
            nrs = tiny.tile([128, NT], F32, tag="nrs")
            nc.vector.tensor_scalar_mul(nrs, rs, -1.0)
            pm2 = tiny.tile([128, 1], F32, tag="pm2")
            nc.vector.tensor_reduce(pm2, nrs, axis=mybir.AxisListType.X, op=ALU.max)
            nlo = tiny.tile([128, 1], F32, tag="nlo")
            nc.gpsimd.partition_all_reduce(nlo, pm2, channels=128,
                                           reduce_op=bass_isa.ReduceOp.max)
            nc.vector.tensor_scalar_mul(lo, nlo, -1.0)
            nc.vector.tensor_scalar_add(hi, hi, 1e-3)

            for it in range(N_BISECT):
                mid = tiny.tile([128, 1], F32, tag="mid")
                nc.vector.tensor_tensor(mid, lo, hi, ALU.add)
                nc.vector.tensor_scalar_mul(mid, mid, 0.5)
                ind = tiny.tile([128, NT], F32, tag="ind")
                pcnt = tiny.tile([128, 1], F32, tag="pcnt")
                nc.vector.tensor_scalar(
                    out=ind, in0=rs, scalar1=mid, scalar2=None, op0=ALU.is_ge,
                    accum_out=pcnt,
                )
                cnt_ps = psum_tp.tile([128, 1], F32, tag="cnt")
                nc.tensor.matmul(cnt_ps, ones_sb, pcnt, start=True, stop=True)
                cge = tiny.tile([128, 1], F32, tag="cge")
                nc.vector.tensor_scalar(out=cge, in0=cnt_ps, scalar1=float(K) - 0.5,
                                        scalar2=None, op0=ALU.is_ge)
                clt = tiny.tile([128, 1], F32, tag="clt")
                nc.vector.tensor_scalar(out=clt, in0=cnt_ps, scalar1=float(K) - 0.5,
                                        scalar2=None, op0=ALU.is_lt)
                nc.vector.copy_predicated(lo, cge, mid)
                nc.vector.copy_predicated(hi, clt, mid)

            mask = persist.tile([128, NT], F32, tag="mask")
            nc.vector.tensor_scalar(out=mask, in0=rs, scalar1=lo, scalar2=None,
                                    op0=ALU.is_ge)

            # ---- LN1 + reversal + masked residual:  x[t] += M[:,t] * (J @ s[15-t]) ----
            def ln_norm(src, dst, dst_dtype_note=None):
                """dst = (src - mean)/sqrt(var+eps), rowwise over free dim."""
                stats = work.tile([128, 2, 6], F32, tag="bnst")
                nc.vector.bn_stats(stats[:, 0, :], src[:, 0:512])
                nc.vector.bn_stats(stats[:, 1, :], src[:, 512:1024])
                mv = work.tile([128, 2], F32, tag="bnmv")
                nc.vector.bn_aggr(mv, stats)
                rstd = work.tile([128, 1], F32, tag="rstd")
                nc.scalar.activation(rstd, mv[:, 1:2], AF.Sqrt, bias=eps_sb, scale=1.0)
                nc.vector.reciprocal(rstd, rstd)
                nc.vector.tensor_scalar(
                    out=dst, in0=src, scalar1=mv[:, 0:1], scalar2=rstd,
                    op0=ALU.subtract, op1=ALU.mult,
                )

            def ln1_s(t):
                s = work.tile([128, D], F32, tag="s")
                ln_norm(x_tiles[t], s)
                if use_g1b1:
                    nc.vector.tensor_tensor(s, s, g1_sb, ALU.mult)
                    nc.vector.tensor_tensor(s, s, b1_sb, ALU.add)
                return s

            def masked_add(t, s_other):
                # x[t] = x[t] + mask[:,t] * (J @ s_other), by 512-halves
                for h in range(2):
                    pr = psum.tile([128, 512], F32, tag="prev")
                    nc.tensor.matmul(pr, J_sb, s_other[:, h * 512:(h + 1) * 512],
                                     start=True, stop=True)
                    nc.vector.scalar_tensor_tensor(
                        out=x_tiles[t][:, h * 512:(h + 1) * 512],
                        in0=pr, scalar=mask[:, t:t + 1],
                        in1=x_tiles[t][:, h * 512:(h + 1) * 512],
                        op0=ALU.mult, op1=ALU.add,
                    )

            for t in range(NT // 2):
                u = NT - 1 - t
                s_t = ln1_s(t)
                s_u = ln1_s(u)
                masked_add(t, s_u)
                masked_add(u, s_t)

            # ---- per block: LN2 -> h2T (bf16, transposed) -> MLP ----
            def do_block(blk):
                t0 = blk * (TOK_BLK // 128)
                h2T = h2pool.tile([128, D // 128, TOK_BLK], BF16, tag="h2T")
                for tt in range(TOK_BLK // 128):
                    t = t0 + tt
                    n2 = work.tile([128, D], F32, tag="n2")
                    ln_norm(x_tiles[t], n2)
                    for kc in range(D // 128):
                        tp = psum_tp.tile([128, 128], F32, tag="tp")
                        nc.tensor.transpose(tp, n2[:, kc * 128:(kc + 1) * 128],
                                            ident_sb)
                        nc.scalar.activation(
                            out=h2T[:, kc, tt * 128:(tt + 1) * 128], in_=tp,
                            func=AF.Identity, bias=ln2b_sb[:, kc:kc + 1],
                            scale=ln2g_sb[:, kc:kc + 1],
                        )

                gT = gpool.tile([128, DF // 128, TOK_BLK], BF16, tag="gT")
                for c in range(DF // 128):
                    fcw_t = wstream.tile([128, D // 128, 128], BF16, tag="fcw")
                    nc.sync.dma_start(
                        fcw_t,
                        fcwT_d.rearrange("(ko p) o -> p ko o", p=128)[
                            :, :, c * 128:(c + 1) * 128],
                    )
                    gp = psum.tile([128, 512], F32, tag="gps")
                    for kc in range(D // 128):
                        nc.tensor.matmul(gp, fcw_t[:, kc, :], h2T[:, kc, :],
                                         start=(kc == 0), stop=(kc == D // 128 - 1))
                    nc.scalar.activation(out=gT[:, c, :], in_=gp,
                                         func=AF.Gelu_apprx_tanh,
                                         bias=fcb_sb[:, c:c + 1], scale=1.0)

                for tt in range(TOK_BLK // 128):
                    t = t0 + tt
                    for h in range(2):
                        yp = psum_y.tile([128, 512], F32, tag="yps")
                        for c in range(DF // 128):
                            nc.tensor.matmul(
                                yp, gT[:, c, tt * 128:(tt + 1) * 128],
                                pwT_sb[:, c, h * 512:(h + 1) * 512],
                                start=(c == 0), stop=(c == DF // 128 - 1))
                        ysb = ypool.tile([128, 512], F32, tag="ysb")
                        nc.vector.scalar_tensor_tensor(
                            out=ysb, in0=yp, scalar=1.0,
                            in1=x_tiles[t][:, h * 512:(h + 1) * 512],
                            op0=ALU.mult, op1=ALU.add,
                        )
                        if use_pb:
                            nc.vector.tensor_tensor(
                                ysb, ysb, pb_sb[:, h * 512:(h + 1) * 512], ALU.add)
                        nc.sync.dma_start(
                            out_d[t * 128:(t + 1) * 128, h * 512:(h + 1) * 512], ysb)

            for blk in (0, 3, 1, 2):
                do_block(blk)

    nc.compile()
    _cached[key] = nc
    return nc


def kernel(**inputs):
    x = np.asarray(inputs["x"], dtype=np.float32)           # [8, 2048, 1024]
    router_w = np.asarray(inputs["router_w"], np.float32)   # [1, 1024]
    ln1_g = np.asarray(inputs["ln1_g"], np.float32)
    ln1_b = np.asarray(inputs["ln1_b"], np.float32)
    ln2_g = np.asarray(inputs["ln2_g"], np.float32)
    ln2_b = np.asarray(inputs["ln2_b"], np.float32)
    fc_w = np.asarray(inputs["fc_w"], np.float32)           # [4096, 1024]
    fc_b = np.asarray(inputs["fc_b"], np.float32)
    proj_w = np.asarray(inputs["proj_w"], np.float32)       # [1024, 4096]
    proj_b = np.asarray(inputs["proj_b"], np.float32)

    use_g1b1 = not (np.all(ln1_g == 1.0) and np.all(ln1_b == 0.0))
    use_pb = bool(np.any(proj_b != 0.0))

    nc = build_program(use_g1b1, use_pb)

    # Host-side layout prep (replication / transpose / bf16 cast of weights).
    rwb = np.ascontiguousarray(np.broadcast_to(router_w[0], (128, D)))
    ln1gb = np.ascontiguousarray(
        np.stack([np.broadcast_to(ln1_g, (128, D)),
                  np.broadcast_to(ln1_b, (128, D))]))
    ln2 = np.ascontiguousarray(np.stack([ln2_g, ln2_b]))
    fcwT = np.ascontiguousarray(fc_w.T).astype(ml_dtypes.bfloat16)
    pwT = np.ascontiguousarray(proj_w.T).astype(ml_dtypes.bfloat16)
    pbb = np.ascontiguousarray(np.broadcast_to(proj_b, (128, D)))
    aux = np.stack([
        np.ones((128, 128), np.float32),
        np.flipud(np.eye(128, dtype=np.float32)),
        np.eye(128, dtype=np.float32),
    ])

    shared = {
        "rwb": rwb, "ln1gb": ln1gb, "ln2": ln2, "fcwT": fcwT,
        "fcb": fc_b, "pwT": pwT, "pbb": pbb, "aux": aux,
    }
    in_maps = [dict(shared, x=np.ascontiguousarray(x[i])) for i in range(B)]

    res = bass_utils.run_bass_kernel_spmd(nc, in_maps, list(range(B)))
    out = np.stack([res.results[i]["out"] for i in range(B)])
    return out.astype(np.float32)


# revision 16
# speedup vs baseline: 1.0620x; 1.0620x over previous
"""Trainium2 Bass kernel for nn_Block_21809843929850 (topk_masking).

Math (after removing dead code in the reference):
  The reference scatters s_out (attention output) into `out` and then
  immediately overwrites the exact same index set with `rev`, so the whole
  q/k/v/attention branch never reaches the output.  What remains is:

    rscore = x @ router_w.T            (router_b shifts all scores equally ->
                                        irrelevant for the top-k *set*)
    M[i,j] = 1 iff rscore[i,j] in top-512 of row i
    h1     = LN(x) * g1 + b1
    xn     = x + M * reverse_seq(h1)        (out[i,j] = M[i,j]*h1[i, L-1-j])
    h2     = LN(xn) * g2 + b2
    y      = xn + gelu_tanh(h2 @ fc_w.T + fc_b) @ proj_w.T + proj_b

Sharding: data-parallel over batch (8 rows -> 8 cores); weights replicated.
Weights for the two MLP matmuls are passed host-transposed+bf16 ([in, out]
layout) so both matmuls contract over the partition dim with no on-device
weight transposes.  All LN/mask/elementwise math runs in fp32.
"""

import sys

sys.path.insert(0, "/opt/trn_rl_repo")

import math

import numpy as np
import ml_dtypes

import concourse.bass as bass
import concourse.mybir as mybir
import concourse.bass_isa as bass_isa
from concourse import bacc
from concourse import bass_utils
from concourse.tile import TileContext

F32 = mybir.dt.float32
BF16 = mybir.dt.bfloat16
AF = mybir.ActivationFunctionType
ALU = mybir.AluOpType

B, L, D = 8, 2048, 1024
DF = 4 * D                     # 4096
K = math.ceil(L * 0.25)        # 512 (top-k size)
NT = L // 128                  # 16 token tiles of 128
TOK_BLK = 512                  # tokens per MLP block
NBLK = L // TOK_BLK            # 4
N_BISECT = 30
EPS = 1e-5

_cached = {}


def build_program(use_g1b1: bool, use_pb: bool):
    import os
    stage = int(os.environ.get("KERNEL_STAGE", "9"))
    key = (use_g1b1, use_pb, stage)
    if key in _cached:
        return _cached[key]

    nc = bacc.Bacc("TRN2", target_bir_lowering=False, debug=False)

    # ---- DRAM I/O ----
    x_d = nc.dram_tensor("x", [L, D], F32, kind="ExternalInput")
    rwb_d = nc.dram_tensor("rwb", [128, D], F32, kind="ExternalInput")
    ln1g_d = nc.dram_tensor("ln1gb", [2, 128, D], F32, kind="ExternalInput")
    ln2_d = nc.dram_tensor("ln2", [2, D], F32, kind="ExternalInput")   # [g;b]
    fcwT_d = nc.dram_tensor("fcwT", [D, DF], BF16, kind="ExternalInput")
    fcb_d = nc.dram_tensor("fcb", [DF], F32, kind="ExternalInput")
    pwT_d = nc.dram_tensor("pwT", [DF, D], BF16, kind="ExternalInput")
    pbb_d = nc.dram_tensor("pbb", [128, D], F32, kind="ExternalInput")
    aux_d = nc.dram_tensor("aux", [3, 128, 128], F32, kind="ExternalInput")
    # aux[0] = ones(128,128), aux[1] = J (anti-diagonal), aux[2] = identity
    out_d = nc.dram_tensor("out", [L, D], F32, kind="ExternalOutput")

    with TileContext(nc) as tc:
        with (
            tc.tile_pool(name="persist", bufs=1) as persist,
            tc.tile_pool(name="xpool", bufs=1) as xpool,
            tc.tile_pool(name="work", bufs=3) as work,
            tc.tile_pool(name="tiny", bufs=4) as tiny,
            tc.tile_pool(name="wstream", bufs=4) as wstream,
            tc.tile_pool(name="gpool", bufs=1) as gpool,
            tc.tile_pool(name="h2pool", bufs=2) as h2pool,
            tc.tile_pool(name="ypool", bufs=3) as ypool,
            tc.tile_pool(name="psum", bufs=2, space="PSUM") as psum,
            tc.tile_pool(name="psum_y", bufs=1, space="PSUM") as psum_y,
            tc.tile_pool(name="psum_tp", bufs=2, space="PSUM") as psum_tp,
        ):
            # ---- persistent small tensors ----
            ones_sb = persist.tile([128, 128], F32, tag="ones")
            nc.sync.dma_start(ones_sb, aux_d[0, :, :])
            J_sb = persist.tile([128, 128], F32, tag="J")
            nc.sync.dma_start(J_sb, aux_d[1, :, :])
            ident_sb = persist.tile([128, 128], F32, tag="ident")
            nc.sync.dma_start(ident_sb, aux_d[2, :, :])
            rwb_sb = persist.tile([128, D], F32, tag="rwb")
            nc.sync.dma_start(rwb_sb, rwb_d[:, :])
            # ln2 g/b as [128, 8] (d = k*128 + p)
            ln2g_sb = persist.tile([128, D // 128], F32, tag="ln2g")
            nc.sync.dma_start(ln2g_sb, ln2_d[0, :].rearrange("(ko p) -> p ko", p=128))
            ln2b_sb = persist.tile([128, D // 128], F32, tag="ln2b")
            nc.sync.dma_start(ln2b_sb, ln2_d[1, :].rearrange("(ko p) -> p ko", p=128))
            fcb_sb = persist.tile([128, DF // 128], F32, tag="fcb")
            nc.sync.dma_start(fcb_sb, fcb_d[:].rearrange("(c p) -> p c", p=128))
            if use_g1b1:
                g1_sb = persist.tile([128, D], F32, tag="g1")
                nc.sync.dma_start(g1_sb, ln1g_d[0, :, :])
                b1_sb = persist.tile([128, D], F32, tag="b1")
                nc.sync.dma_start(b1_sb, ln1g_d[1, :, :])
            if use_pb:
                pb_sb = persist.tile([128, D], F32, tag="pb")
                nc.sync.dma_start(pb_sb, pbb_d[:, :])
            eps_sb = persist.tile([128, 1], F32, tag="eps")
            nc.vector.memset(eps_sb, EPS)

            pwT_view = pwT_d[:, :].rearrange("(c p) o -> p c o", p=128)

            # ---- load x ----
            x_tiles = []
            for t in range(NT):
                xt = xpool.tile([128, D], F32, tag=f"x{t}")
                nc.sync.dma_start(xt, x_d[t * 128:(t + 1) * 128, :])
                x_tiles.append(xt)

            # ---- router scores ----
            rs = persist.tile([128, NT], F32, tag="rs")
            if stage == 0:
                nc.vector.memset(rs, 0.5)
            for t in range(NT) if stage >= 1 else []:
                trash = work.tile([128, D], F32, tag="rtrash")
                nc.vector.tensor_tensor(trash, x_tiles[t], rwb_sb, ALU.mult)
                nc.vector.tensor_reduce(rs[:, t:t + 1], trash,
                                        axis=mybir.AxisListType.X, op=ALU.add)

            # ---- top-k threshold by bisection ----
            lo = persist.tile([128, 1], F32, tag="lo")
            hi = persist.tile([128, 1], F32, tag="hi")
            if stage == 0:
                nc.vector.memset(lo, 0.0)
                nc.vector.memset(hi, 1.0)
            import os as _os
            if stage == 0:
                pass
            elif _os.environ.get("KERNEL_NO_PAR"):
                nc.vector.memset(lo, -30.0)
                nc.vector.memset(hi, 30.0)
            else:
                pm = tiny.tile([128, 1], F32, tag="pm")
                nc.vector.tensor_reduce(pm, rs, axis=mybir.AxisListType.X, op=ALU.max)
                nc.gpsimd.partition_all_reduce(hi, pm, channels=128,
                                               reduce_op=bass_isa.ReduceOp.max)
                nrs = tiny.tile([128, NT], F32, tag="nrs")
                nc.vector.tensor_scalar_mul(nrs, rs, -1.0)
                pm2 = tiny.tile([128, 1], F32, tag="pm2")
                nc.vector.tensor_reduce(pm2, nrs, axis=mybir.AxisListType.X, op=ALU.max)
                nlo = tiny.tile([128, 1], F32, tag="nlo")
                nc.gpsimd.partition_all_reduce(nlo, pm2, channels=128,
                                               reduce_op=bass_isa.ReduceOp.max)
                nc.vector.tensor_scalar_mul(lo, nlo, -1.0)
                nc.vector.tensor_scalar_add(hi, hi, 1e-3)

            for it in range(N_BISECT) if (stage >= 1 and not _os.environ.get('KERNEL_NO_BISECT')) else []:
                mid = tiny.tile([128, 1], F32, tag="mid")
                nc.vector.tensor_tensor(mid, lo, hi, ALU.add)
                nc.vector.tensor_scalar_mul(mid, mid, 0.5)
                ind = tiny.tile([128, NT], F32, tag="ind")
                pcnt = tiny.tile([128, 1], F32, tag="pcnt")
                nc.vector.tensor_scalar(
                    out=ind, in0=rs, scalar1=mid, scalar2=None, op0=ALU.is_ge,
                )
                nc.vector.tensor_reduce(pcnt, ind, axis=mybir.AxisListType.X,
                                        op=ALU.add)
                cnt_ps = psum_tp.tile([128, 1], F32, tag="tp")
                nc.tensor.matmul(cnt_ps, ones_sb, pcnt, start=True, stop=True)
                cge = tiny.tile([128, 1], mybir.dt.uint8, tag="cge")
                nc.vector.tensor_scalar(out=cge, in0=cnt_ps, scalar1=float(K) - 0.5,
                                        scalar2=None, op0=ALU.is_ge)
                clt = tiny.tile([128, 1], mybir.dt.uint8, tag="clt")
                nc.vector.tensor_scalar(out=clt, in0=cnt_ps, scalar1=float(K) - 0.5,
                                        scalar2=None, op0=ALU.is_lt)
                nc.vector.copy_predicated(lo, cge, mid)
                nc.vector.copy_predicated(hi, clt, mid)

            mask = persist.tile([128, NT], F32, tag="mask")
            nc.vector.tensor_scalar(out=mask, in0=rs, scalar1=lo, scalar2=None,
                                    op0=ALU.is_ge)
            if stage <= 1:
                nc.sync.dma_start(out_d[0:128, 0:NT], mask)
                for t in range(NT):
                    nc.sync.dma_start(out_d[t * 128:(t + 1) * 128, NT:D], x_tiles[t][:, NT:D])

            # ---- LN1 + reversal + masked residual:  x[t] += M[:,t] * (J @ s[15-t]) ----
            def ln_norm(src, dst, dst_dtype_note=None):
                """dst = (src - mean)/sqrt(var+eps), rowwise over free dim."""
                stats = work.tile([128, 2, 6], F32, tag="bnst")
                nc.vector.bn_stats(stats[:, 0, :], src[:, 0:512])
                nc.vector.bn_stats(stats[:, 1, :], src[:, 512:1024])
                mv = work.tile([128, 2], F32, tag="bnmv")
                nc.vector.bn_aggr(mv, stats)
                rstd = work.tile([128, 1], F32, tag="rstd")
                nc.scalar.activation(rstd, mv[:, 1:2], AF.Sqrt, bias=eps_sb, scale=1.0)
                nc.vector.reciprocal(rstd, rstd)
                nc.vector.tensor_scalar(
                    out=dst, in0=src, scalar1=mv[:, 0:1], scalar2=rstd,
                    op0=ALU.subtract, op1=ALU.mult,
                )

            def ln1_s(t):
                s = work.tile([128, D], F32, tag="s")
                ln_norm(x_tiles[t], s)
                if use_g1b1:
                    nc.vector.tensor_tensor(s, s, g1_sb, ALU.mult)
                    nc.vector.tensor_tensor(s, s, b1_sb, ALU.add)
                return s

            def masked_add(t, s_other):
                # x[t] = x[t] + mask[:,t] * (J @ s_other), by 512-halves
                for h in range(2):
                    pr = psum_tp.tile([128, 512], F32, tag="tp")
                    nc.tensor.matmul(pr, J_sb, s_other[:, h * 512:(h + 1) * 512],
                                     start=True, stop=True)
                    nc.vector.scalar_tensor_tensor(
                        out=x_tiles[t][:, h * 512:(h + 1) * 512],
                        in0=pr, scalar=mask[:, t:t + 1],
                        in1=x_tiles[t][:, h * 512:(h + 1) * 512],
                        op0=ALU.mult, op1=ALU.add,
                    )

            if stage >= 2:
              for t in range(NT // 2):
                u = NT - 1 - t
                s_t = ln1_s(t)
                s_u = ln1_s(u)
                masked_add(t, s_u)
                masked_add(u, s_t)
            if stage == 2:
                for t in range(NT):
                    nc.sync.dma_start(out_d[t * 128:(t + 1) * 128, :], x_tiles[t])

            # ---- per block: LN2 -> h2T (bf16, transposed) -> MLP ----
            def do_block(blk):
                t0 = blk * (TOK_BLK // 128)
                h2T = h2pool.tile([128, D // 128, TOK_BLK], BF16, tag="h2T")
                for tt in range(TOK_BLK // 128):
                    t = t0 + tt
                    n2 = work.tile([128, D], F32, tag="n2")
                    ln_norm(x_tiles[t], n2)
                    for kc in range(D // 128):
                        tp = psum_tp.tile([128, 512], F32, tag="tp", name="tp")[:, :128]
                        nc.tensor.transpose(tp, n2[:, kc * 128:(kc + 1) * 128],
                                            ident_sb)
                        nc.scalar.activation(
                            out=h2T[:, kc, tt * 128:(tt + 1) * 128], in_=tp,
                            func=AF.Identity, bias=ln2b_sb[:, kc:kc + 1],
                            scale=ln2g_sb[:, kc:kc + 1],
                        )

                if stage <= 3:
                    return
                gT = gpool.tile([128, DF // 128, TOK_BLK], BF16, tag="gT")
                for c in range(DF // 128):
                    fcw_t = wstream.tile([128, D // 128, 128], BF16, tag="fcw")
                    nc.sync.dma_start(
                        fcw_t,
                        fcwT_d[:, :].rearrange("(ko p) o -> p ko o", p=128)[
                            :, :, c * 128:(c + 1) * 128],
                    )
                    gp = psum.tile([128, 512], F32, tag="gps")
                    for kc in range(D // 128):
                        nc.tensor.matmul(gp, fcw_t[:, kc, :], h2T[:, kc, :],
                                         start=(kc == 0), stop=(kc == D // 128 - 1))
                    nc.scalar.activation(out=gT[:, c, :], in_=gp,
                                         func=AF.Gelu_apprx_tanh,
                                         bias=fcb_sb[:, c:c + 1], scale=1.0)

                if stage <= 4:
                    return
                for h in range(2):
                    yps = []
                    for tt in range(TOK_BLK // 128):
                        yp = psum_y.tile([128, 512], F32, tag=f"yps{tt}", name=f"yps{tt}")
                        yps.append(yp)
                    for c in range(DF // 128):
                        pw_t = wstream.tile([128, 512], BF16, tag="pw")
                        nc.sync.dma_start(
                            pw_t, pwT_view[:, c, h * 512:(h + 1) * 512])
                        for tt in range(TOK_BLK // 128):
                            nc.tensor.matmul(
                                yps[tt], gT[:, c, tt * 128:(tt + 1) * 128], pw_t,
                                start=(c == 0), stop=(c == DF // 128 - 1))
                    for tt in range(TOK_BLK // 128):
                        t = t0 + tt
                        ysb = ypool.tile([128, 512], F32, tag="ysb")
                        nc.vector.scalar_tensor_tensor(
                            out=ysb, in0=yps[tt], scalar=1.0,
                            in1=x_tiles[t][:, h * 512:(h + 1) * 512],
                            op0=ALU.mult, op1=ALU.add,
                        )
                        if use_pb:
                            nc.vector.tensor_tensor(
                                ysb, ysb, pb_sb[:, h * 512:(h + 1) * 512], ALU.add)
                        nc.sync.dma_start(
                            out_d[t * 128:(t + 1) * 128, h * 512:(h + 1) * 512], ysb)

            if stage >= 3:
                for blk in (0, 3, 1, 2):
                    do_block(blk)

    nc.compile()
    _cached[key] = nc
    return nc


def kernel(**inputs):
    x = np.asarray(inputs["x"], dtype=np.float32)           # [8, 2048, 1024]
    router_w = np.asarray(inputs["router_w"], np.float32)   # [1, 1024]
    ln1_g = np.asarray(inputs["ln1_g"], np.float32)
    ln1_b = np.asarray(inputs["ln1_b"], np.float32)
    ln2_g = np.asarray(inputs["ln2_g"], np.float32)
    ln2_b = np.asarray(inputs["ln2_b"], np.float32)
    fc_w = np.asarray(inputs["fc_w"], np.float32)           # [4096, 1024]
    fc_b = np.asarray(inputs["fc_b"], np.float32)
    proj_w = np.asarray(inputs["proj_w"], np.float32)       # [1024, 4096]
    proj_b = np.asarray(inputs["proj_b"], np.float32)

    use_g1b1 = not (np.all(ln1_g == 1.0) and np.all(ln1_b == 0.0))
    use_pb = bool(np.any(proj_b != 0.0))

    nc = build_program(use_g1b1, use_pb)
    in_maps = prep_in_maps(inputs)
    res = bass_utils.run_bass_kernel_spmd(nc, in_maps, list(range(B)))
    out = np.stack([res.results[i]["out"] for i in range(B)])
    return out.astype(np.float32)


def prep_in_maps(inputs):
    x = np.asarray(inputs["x"], dtype=np.float32)
    router_w = np.asarray(inputs["router_w"], np.float32)
    ln1_g = np.asarray(inputs["ln1_g"], np.float32)
    ln1_b = np.asarray(inputs["ln1_b"], np.float32)
    ln2_g = np.asarray(inputs["ln2_g"], np.float32)
    ln2_b = np.asarray(inputs["ln2_b"], np.float32)
    fc_w = np.asarray(inputs["fc_w"], np.float32)
    fc_b = np.asarray(inputs["fc_b"], np.float32)
    proj_w = np.asarray(inputs["proj_w"], np.float32)
    proj_b = np.asarray(inputs["proj_b"], np.float32)

    # Host-side layout prep (replication / transpose / bf16 cast of weights).
    rwb = np.ascontiguousarray(np.broadcast_to(router_w[0], (128, D)))
    ln1gb = np.ascontiguousarray(
        np.stack([np.broadcast_to(ln1_g, (128, D)),
                  np.broadcast_to(ln1_b, (128, D))]))
    ln2 = np.ascontiguousarray(np.stack([ln2_g, ln2_b]))
    fcwT = np.ascontiguousarray(fc_w.T).astype(ml_dtypes.bfloat16)
    pwT = np.ascontiguousarray(proj_w.T).astype(ml_dtypes.bfloat16)
    pbb = np.ascontiguousarray(np.broadcast_to(proj_b, (128, D)))
    aux = np.stack([
        np.ones((128, 128), np.float32),
        np.flipud(np.eye(128, dtype=np.float32)),
        np.eye(128, dtype=np.float32),
    ])

    shared = {
        "rwb": rwb, "ln1gb": ln1gb, "ln2": ln2, "fcwT": fcwT,
        "fcb": fc_b, "pwT": pwT, "pbb": pbb, "aux": aux,
    }
    return [dict(shared, x=np.ascontiguousarray(x[i])) for i in range(B)]
